# revision 25
# baseline (speedup 1.0000x reference)
"""Trainium2 Bass kernel for CriterionMiniBatchCrossImagePair.

Computes: prep = L2norm_C(avgpool4x4(x)) per image -> all BxB pairwise
[N,N] similarity maps for S and T -> KL(softmax_T || softmax_S) batchmean.

Sharding: 8 cores. Each core preps ONE of the 8 images (4 S + 4 T),
AllGathers the prepped fp8e4 features (two position-halves, pipelined so
the first AllGather overlaps the second prep half and unit compute
overlaps the second AllGather), then computes 2 of the 16 (i,j) pairs
(transpose-pairing so each core touches only 2 image indices). Scalar
partials are summed on the host.

Math used per row-block (row softmax over m):
  KL_row = sum_m p_t*(log p_t - log p_s)
         = (1/T) * (sum_m eT*(rawT-rawS)) / Z_T - ln Z_T + ln Z_S
with eT = exp(rawT/T), Z_X = sum_m exp(rawX/T). No max-subtraction
needed: raw in [-1,1] so raw/T in [-10,10].

The (rawT - rawS) difference map is produced directly in PSUM by the PE
(accumulate +T then -S using a negated stationary copy), so the vector
engines only run one multiply-accumulate pass per unit. Features are
stored [128, 2, pos] (channel chunk on dim 1) which is exactly the fp8
DoubleRow matmul layout: one matmul contracts all 256 channels.
"""

import numpy as np

import concourse.bass as bass
import concourse.mybir as mybir
import concourse.tile as tile
from concourse.bass_utils import run_bass_kernel_spmd

F32 = mybir.dt.float32
BF16 = mybir.dt.bfloat16
FP8 = mybir.dt.float8e4
I32 = mybir.dt.int32
AF = mybir.ActivationFunctionType
ALU = mybir.AluOpType
PM = mybir.MatmulPerfMode

TEMPERATURE = 0.1
B, C, H, W = 4, 256, 128, 128
PATCH = 4
PH, PW = H // PATCH, W // PATCH  # 32 x 32
N = PH * PW  # 1024
NCORES = 8
CC = C // 128  # channel chunks of 128
FB = 4  # h-row chunks per channel chunk (32 h rows each)
HROWS = H // FB  # 32
NBLK = N // 128  # 8 row blocks per pair
NPAIR = 2  # pairs per core
NH = N // 2  # 512 positions per half

# core -> [(n0, m0), (n1, m1)] image-index pairs (covers all 16 (i,j))
PAIRS_PER_CORE = [
    [(0, 0), (1, 1)],
    [(2, 2), (3, 3)],
    [(0, 1), (1, 0)],
    [(2, 3), (3, 2)],
    [(0, 2), (2, 0)],
    [(1, 3), (3, 1)],
    [(0, 3), (3, 0)],
    [(1, 2), (2, 1)],
]


def legalize_waits(nc):
    """Split multi-wait instructions into single-wait NoOps.

    The walrus build in this environment encodes at most one sync-wait per
    instruction (and none on register-offset pseudo DMAs): anything more dies
    in codegen with "Too many sync wait commands". Semantically, hoisting a
    wait onto a NoOp immediately before the instruction on the same engine
    stream is identical (both block the engine's sequencer).
    """
    n_id = 0
    for f in nc.m.functions:
        for b in f.blocks:
            lst = b.instructions
            out = []
            changed = False
            for ins in lst:
                si = ins.sync_info
                waits = list(si.on_wait) if si and si.on_wait else []
                keep = 0 if isinstance(ins, mybir.InstDMACopy) else 1
                if len(waits) > keep:
                    moved, kept = waits[: len(waits) - keep], waits[len(waits) - keep :]
                    for w in moved:
                        nop = mybir.InstNoOp(name=f"waitnop_{n_id}")
                        n_id += 1
                        nop.engine = ins.engine
                        nop.sync_info = mybir.SyncInfo(on_wait=[w], on_update=[])
                        out.append(nop)
                    ins.sync_info = mybir.SyncInfo(
                        on_wait=kept, on_update=list(si.on_update)
                    )
                    changed = True
                out.append(ins)
            if changed:
                b.instructions = out
    return nc


def build_bass():
    nc = bass.Bass(num_devices=NCORES)

    img = nc.declare_dram_parameter("img", [C, H, W], F32, isOutput=False)
    sel = nc.declare_dram_parameter("sel", [1, 8], I32, isOutput=False)
    out_partial = nc.declare_dram_parameter("out_partial", [1, 1], F32, isOutput=True)

    with tile.TileContext(nc) as tc:
        with (
            tc.tile_pool(name="dram", bufs=1, space="DRAM") as dpool,
            tc.tile_pool(name="consts", bufs=1) as cpool,
        ):
            ag_in_h = [
                dpool.tile([128, CC, NH], FP8, name=f"ag_in{h}") for h in range(2)
            ]
            ag_out_h = [
                dpool.tile(
                    [NCORES, 128, CC, NH], FP8, addr_space="Shared",
                    name=f"ag_out{h}",
                )
                for h in range(2)
            ]
            F32R = mybir.dt.float32r
            ones_col = cpool.tile([128, 1], F32)
            nc.vector.memset(ones_col[:], 1.0)
            ones_row = cpool.tile([1, 128], F32)
            nc.vector.memset(ones_row[:], 1.0)

            # ---------------- Stage A: prep own image ----------------
            with (
                tc.tile_pool(name="prep", bufs=2) as ppool,
                tc.tile_pool(name="prep_ps", bufs=1, space="PSUM") as pspool,
                tc.tile_pool(name="prep_keep", bufs=1) as kpool,
            ):
                u = kpool.tile([128, CC, PH, PW], F32)  # pooled (unnormalized)
                ss_ps = pspool.tile([1, N], F32)  # sum_c u^2
                NQ = N // 4  # 256 positions per fb-quarter
                PB = PH // 4  # 8 pooled rows per fb-quarter
                # half-major: columns n in [half*512,(half+1)*512) finish
                # (pool+norm+quantize) and AllGather while the other half
                # still preps; norm/quantize runs quarter-wise so the
                # AllGather launches right after the half's last DMA chunk
                for half in range(2):
                    feat8 = kpool.tile([128, CC, NH], FP8, name=f"feat8h{half}")
                    for fb in (2 * half, 2 * half + 1):
                        for cc in range(CC):
                            raw = ppool.tile([128, HROWS, W], F32, tag="raw", bufs=3)
                            nc.sync.dma_start(
                                raw[:],
                                img[cc * 128 : (cc + 1) * 128, fb * HROWS : (fb + 1) * HROWS, :],
                            )
                            # leaf adds off the serial spine go to Pool
                            wp1 = ppool.tile([128, HROWS, PW], F32, tag="wp1")
                            wp2 = ppool.tile([128, HROWS, PW], F32, tag="wp2")
                            wp = ppool.tile([128, HROWS, PW], F32, tag="wp")
                            nc.vector.tensor_add(wp1[:], raw[:, :, 0::4], raw[:, :, 1::4])
                            nc.gpsimd.tensor_add(wp2[:], raw[:, :, 2::4], raw[:, :, 3::4])
                            nc.vector.tensor_add(wp[:], wp1[:], wp2[:])
                            hp1 = ppool.tile([128, HROWS // 4, PW], F32, tag="hp1")
                            hp2 = ppool.tile([128, HROWS // 4, PW], F32, tag="hp2")
                            nc.gpsimd.tensor_add(hp1[:], wp[:, 0::4, :], wp[:, 1::4, :])
                            nc.vector.tensor_add(hp2[:], wp[:, 2::4, :], wp[:, 3::4, :])
                            nc.vector.tensor_add(
                                u[:, cc, fb * PB : (fb + 1) * PB, :],
                                hp1[:],
                                hp2[:],
                            )
                            # sq immediately behind this chunk's pooling on
                            # the DVE queue (keeps the AllGather launch chain
                            # short after the half's last DMA chunk)
                            sq = ppool.tile([128, NQ], F32, tag="sq")
                            uq = u[:, cc, fb * PB : (fb + 1) * PB, :].rearrange("p a b -> p (a b)")
                            nc.vector.tensor_mul(sq[:], uq, uq)
                            nc.tensor.matmul(
                                ss_ps[:, fb * NQ : (fb + 1) * NQ],
                                ones_col[:],
                                sq[:],
                                start=(cc == 0),
                                stop=(cc == CC - 1),
                            )
                        lnss = ppool.tile([1, NQ], F32, tag="lnss")
                        nc.scalar.activation(lnss[:], ss_ps[:, fb * NQ : (fb + 1) * NQ], AF.Ln)
                        inv = ppool.tile([1, NQ], F32, tag="inv")
                        nc.scalar.activation(inv[:], lnss[:], AF.Exp, scale=-0.5)
                        inv_b = pspool.tile([128, NQ], F32, tag="inv_b", bufs=2)
                        nc.tensor.matmul(inv_b[:], ones_row[:], inv[:], start=True, stop=True)
                        q0 = (fb - 2 * half) * NQ
                        for cc in range(CC):
                            uq = u[:, cc, fb * PB : (fb + 1) * PB, :].rearrange("p a b -> p (a b)")
                            nc.vector.tensor_mul(feat8[:, cc, q0 : q0 + NQ], uq, inv_b[:])
                    nc.sync.dma_start(ag_in_h[half][:], feat8[:])
                    nc.gpsimd.collective_compute(
                        "AllGather",
                        ALU.bypass,
                        replica_groups=[list(range(NCORES))],
                        ins=[ag_in_h[half].opt()],
                        outs=[ag_out_h[half].opt()],
                    )

            # ---------------- Stage B: 2 pairs of similarity maps ----------------
            with (
                tc.tile_pool(name="slots", bufs=1) as spool,
                tc.tile_pool(name="acc", bufs=1) as apool,
                tc.tile_pool(name="work", bufs=4) as wpool,
            ):
                sel_sb = apool.tile([1, 8], I32)
                nc.sync.dma_start(sel_sb[:], sel[:])

                # slot order: NS0, MS0, NS1, MS1, NT0, MT0, NT1, MT1
                # slots[s][half] is a [128, CC, 512] fp8 tile
                selv = [nc.sync.value_load(sel_sb[0:1, s : s + 1]) for s in range(8)]
                slots = [[None, None] for _ in range(8)]

                negNS = [[None, None] for _ in range(NPAIR)]

                def load_half_slots(half, order, neg):
                    for s in order:
                        t = spool.tile([128, CC, NH], FP8, name=f"slot{s}h{half}")
                        nc.sync.dma_start(
                            t[:], ag_out_h[half][bass.ds(selv[s], 1)].squeeze(0)
                        )
                        slots[s][half] = t
                        if neg and s in (0, 2):
                            # negated S stationary for the PE-side D = T - S
                            p = s // 2
                            nt = spool.tile([128, CC, NH], FP8, name=f"negNS{p}h{half}")
                            nc.vector.tensor_scalar_mul(nt[:], t[:], -1.0)
                            negNS[p][half] = nt

                # DMA order matches consumption order: h0 units want pair-0's
                # four tiles first; the first post-AG1 units want the h1
                # stationary tiles (NS*/NT*) before the h1 moving tiles
                load_half_slots(0, [0, 1, 4, 5, 2, 3, 6, 7], neg=True)
                load_half_slots(1, [0, 4, 2, 6, 1, 5, 3, 7], neg=True)

                # flat accumulators, col = h*16 + p*8 + nb
                zT = apool.tile([128, 32], F32)
                zS = apool.tile([128, 32], F32)
                racc = apool.tile([128, 32], F32)

                with (
                    tc.tile_pool(name="sims_ps", bufs=3, space="PSUM") as simspool,
                    tc.tile_pool(name="simd_ps", bufs=2, space="PSUM") as simdpool,
                ):
                    def unit(h, p, nb):
                        col = h * 16 + p * 8 + nb
                        nh_, nloc = nb // 4, (nb % 4) * 128
                        ns = slots[2 * p][nh_][:, :, nloc : nloc + 128]
                        ms = slots[2 * p + 1][h][:]
                        nt = slots[4 + 2 * p][nh_][:, :, nloc : nloc + 128]
                        mt = slots[4 + 2 * p + 1][h][:]
                        nn = negNS[p][nh_][:, :, nloc : nloc + 128]
                        ps = simspool.tile([128, 2, NH], F32, tag="ps")
                        psd = simdpool.tile([128, NH], F32, tag="psd")
                        nc.tensor.matmul(
                            ps[:, 0, :], nt, mt, start=True, stop=True,
                            perf_mode=PM.DoubleRow,
                        )
                        nc.tensor.matmul(
                            ps[:, 1, :], ns, ms, start=True, stop=True,
                            perf_mode=PM.DoubleRow,
                        )
                        nc.tensor.matmul(
                            psd[:], nt, mt, start=True, stop=False,
                            perf_mode=PM.DoubleRow,
                        )
                        nc.tensor.matmul(
                            psd[:], nn, ms, start=False, stop=True,
                            perf_mode=PM.DoubleRow,
                        )
                        eTS = wpool.tile([128, 2 * NH], BF16, tag="eTS")
                        nc.scalar.activation(
                            eTS[:],
                            ps[:, 0:2, :].rearrange("p a b -> p (a b)"),
                            AF.Exp,
                            scale=1.0 / TEMPERATURE,
                        )
                        jzT = wpool.tile([128, NH], BF16, tag="jzT")
                        jzS = wpool.tile([128, NH], BF16, tag="jzS")
                        jD = wpool.tile([128, NH], BF16, tag="jD")
                        nc.vector.tensor_scalar(
                            out=jzT[:], in0=eTS[:, :NH], scalar1=0.0, scalar2=0.0,
                            op0=ALU.add, op1=ALU.add, accum_out=zT[:, col : col + 1],
                        )
                        nc.vector.tensor_scalar(
                            out=jzS[:], in0=eTS[:, NH:], scalar1=0.0, scalar2=0.0,
                            op0=ALU.add, op1=ALU.add, accum_out=zS[:, col : col + 1],
                        )
                        nc.vector.scalar_tensor_tensor(
                            out=jD[:], in0=eTS[:, :NH], scalar=1.0, in1=psd[:],
                            op0=ALU.mult, op1=ALU.mult,
                            accum_out=racc[:, col : col + 1],
                        )

                    # units whose inputs all come from AllGather 0 first (they
                    # overlap AllGather 1); the tail is pair-major so pair 0's
                    # combine overlaps pair 1's units
                    for p in range(NPAIR):
                        for nb in range(4):
                            unit(0, p, nb)
                    for p in range(NPAIR):
                        for nb in range(4, NBLK):
                            unit(0, p, nb)
                    for p in range(NPAIR):
                        for nb in range(NBLK):
                            unit(1, p, nb)

                # ---------------- final combine ----------------
                zTt = apool.tile([128, 16], F32)
                zSt = apool.tile([128, 16], F32)
                rt = apool.tile([128, 16], F32)
                nc.vector.tensor_add(zTt[:], zT[:, :16], zT[:, 16:])
                nc.gpsimd.tensor_add(zSt[:], zS[:, :16], zS[:, 16:])
                nc.vector.tensor_add(rt[:], racc[:, :16], racc[:, 16:])
                recT = apool.tile([128, 16], F32)
                nc.vector.reciprocal(recT[:], zTt[:])
                kl1 = apool.tile([128, 16], F32)
                nc.vector.scalar_tensor_tensor(
                    out=kl1[:], in0=rt[:], scalar=1.0 / TEMPERATURE, in1=recT[:],
                    op0=ALU.mult, op1=ALU.mult,
                )
                lnT = apool.tile([128, 16], F32)
                nc.scalar.activation(lnT[:], zTt[:], AF.Ln)
                lnS = apool.tile([128, 16], F32)
                nc.scalar.activation(lnS[:], zSt[:], AF.Ln)
                lnD = apool.tile([128, 16], F32)
                nc.gpsimd.tensor_sub(lnD[:], lnS[:], lnT[:])
                kl3 = apool.tile([128, 16], F32)
                nc.vector.tensor_add(kl3[:], kl1[:], lnD[:])
                klsum = apool.tile([128, 1], F32)
                nc.vector.reduce_sum(klsum[:], kl3[:], axis=mybir.AxisListType.X)
                scaled = apool.tile([128, 1], F32)
                nc.scalar.mul(scaled[:], klsum[:], 1.0 / (N * B * B))
                with tc.tile_pool(name="tot_ps", bufs=1, space="PSUM") as tpool:
                    tot_ps = tpool.tile([1, 1], F32)
                    nc.tensor.matmul(tot_ps[:], scaled[:], ones_col[:], start=True, stop=True)
                    outsb = apool.tile([1, 1], F32)
                    nc.scalar.copy(outsb[:], tot_ps[:])
                    nc.sync.dma_start(out_partial[:], outsb[:])

    return nc


_NC_CACHE = None


def _get_nc():
    global _NC_CACHE
    if _NC_CACHE is None:
        _NC_CACHE = legalize_waits(build_bass())
    return _NC_CACHE


def make_in_maps(feat_S, feat_T):
    feat_S = np.asarray(feat_S, dtype=np.float32)
    feat_T = np.asarray(feat_T, dtype=np.float32)
    in_maps = []
    for c in range(NCORES):
        img = feat_S[c] if c < B else feat_T[c - B]
        prs = PAIRS_PER_CORE[c]
        selv = [prs[0][0], prs[0][1], prs[1][0], prs[1][1]]
        selv = selv + [x + B for x in selv]
        in_maps.append(
            {
                "img": np.ascontiguousarray(img),
                "sel": np.asarray(selv, dtype=np.int32).reshape(1, 8),
            }
        )
    return in_maps


def run(feat_S, feat_T, **run_kwargs):
    nc = _get_nc()
    in_maps = make_in_maps(feat_S, feat_T)
    res = run_bass_kernel_spmd(nc, in_maps, core_ids=list(range(NCORES)), **run_kwargs)
    total = np.float32(0.0)
    for r in res.results:
        total += np.float32(r["out_partial"].reshape(()))
    return np.asarray(total, dtype=np.float32), res


def kernel(**inputs):
    out, _ = run(inputs["feat_S"], inputs["feat_T"])
    return out
